# revision 38
# baseline (speedup 1.0000x reference)
"""Trainium2 Bass kernel for nn_BinarizedLinear (ES population binary matvec).

Computes, for each direction d: out[d, o] = (sum_i W[d,o,i] * x[d,i]) > bias[d,o]
with W in {0,1} (f32), x in {0,1} (bool), bias f32.

Hybrid strategy (memory-bound problem -> shrink the stream, use every engine):
  - 8 directions per core.  Five stream as 1-bit-packed uint16 words
    (0.5 MiB/dir) and are popcounted on the DVE with the classic SWAR
    ladder (bitwise ops are raw bits; add/sub run through the fp32-internal
    ALU and stay exact below 2^24).  Three stream as fp8 (4 MiB/dir) and
    run as matvecs on the otherwise-idle PE: W^T is the moving operand,
    x sits as a [128, 3] stationary whose only nonzero column is the
    direction index, so each direction's activations accumulate into its
    own PSUM partition -> the compare + store are contiguous, no transpose.
    The 5/3 split balances DVE time against DMA bytes (PE fp8 runs at
    bf16 speed; products are 0/1 and PSUM accumulates fp32-exact).
  - SWAR ladder per direction (uint16 elements, 2048 els/partition per
    instruction -- larger flat instructions drop below the 2x/4x DVE modes):
       u  = w & x
       v1 = u - ((u >> 1) & 0x5555)              crumb pops <= 2
       v2 = (v1 & 0x3333) + ((v1>>2) & 0x3333)   nibble pops <= 4
    then a fold tree along each row's 128 words: fold64, nibble->byte
    merge via masked adds, fold to 8, byte->word merge, fold to 1 (the
    final add emits fp32).  PE-transpose, is_gt vs bias, store bools.
"""

from contextlib import ExitStack

import numpy as np

import concourse.bass as bass
from concourse import mybir
from concourse.bass_utils import run_bass_kernel_spmd

N_CORES = 8
D_TOT, OUT, IN = 64, 2048, 2048
D = D_TOT // N_CORES  # 8 directions per core
DS = 5                # SWAR (bit-packed) directions per core
DP = D - DS           # PE (fp8 matvec) directions per core
P = 128
NT = OUT // P         # 16 o-tiles per direction
K = IN // 16          # 128 packed uint16 words per row
DT2 = DS * NT         # 80 (dir, tile) columns on the SWAR path
KT = IN // P          # 16 contraction tiles per PE direction
PC_KT = 2             # K-tiles per PE weight DMA piece
NPIECE = DP * KT // PC_KT  # 24 pieces
NSLOT = 4             # fp8 weight ring slots
NCH = 4               # 512-wide PSUM chunks per row


def build_program() -> bass.Bass:
    f32 = mybir.dt.float32
    u16 = mybir.dt.uint16
    u8 = mybir.dt.uint8
    f8 = mybir.dt.float8e4
    Alu = mybir.AluOpType

    nc = bass.Bass()
    w = nc.declare_dram_parameter("w", [DS, P, NT, K], u16, isOutput=False)
    x = nc.declare_dram_parameter("x", [P, DS, K], u16, isOutput=False)
    b = nc.declare_dram_parameter("b", [DS, OUT], f32, isOutput=False)
    wpe = nc.declare_dram_parameter(
        "wpe", [DP, KT, P, OUT], f8, isOutput=False
    )
    xpe = nc.declare_dram_parameter("xpe", [P, DP, KT, 16], f8, isOutput=False)
    bpe = nc.declare_dram_parameter("bpe", [DP, OUT], f32, isOutput=False)
    o = nc.declare_dram_parameter("o", [D, OUT], u8, isOutput=True)

    # [80, 128] views of SWAR bias/out matching the post-transpose layout:
    # partition j = d*16 + c, free f = p  ->  flat offset j*128 + f.
    bias_r = b[:].rearrange("d (h f) -> (d h) f", f=P)
    out_r = o[0:DS, :].rearrange("d (h f) -> (d h) f", f=P)
    ope_r = o[DS:D, :]

    psum_t = nc.alloc_psum_tensor("psum_t", [DT2, P], f32)
    psum_pe = nc.alloc_psum_tensor("psum_pe", [16, OUT], f32)

    with ExitStack() as ctx:
        wsb = ctx.enter_context(nc.sbuf_tensor("wsb", [P, DS, NT, K], u16))
        xb = ctx.enter_context(nc.sbuf_tensor("xb", [P, DS, K], u16))
        u_s = ctx.enter_context(nc.sbuf_tensor("u_s", [P, NT, K], u16))
        t_s = ctx.enter_context(nc.sbuf_tensor("t_s", [P, NT, K], u16))
        v1_s = ctx.enter_context(nc.sbuf_tensor("v1_s", [P, NT, K], u16))
        t2_s = ctx.enter_context(nc.sbuf_tensor("t2_s", [P, NT, K], u16))
        m_s = ctx.enter_context(nc.sbuf_tensor("m_s", [P, NT, K], u16))
        v2 = ctx.enter_context(nc.sbuf_tensor("v2", [P, DT2, K], u16))
        f1 = ctx.enter_context(nc.sbuf_tensor("f1", [P, DT2, 64], u16))
        h_s = ctx.enter_context(nc.sbuf_tensor("h_s", [P, DT2, 64], u16))
        g_s = ctx.enter_context(nc.sbuf_tensor("g_s", [P, DT2, 64], u16))
        g2 = ctx.enter_context(nc.sbuf_tensor("g2", [P, DT2, 32], u16))
        g3 = ctx.enter_context(nc.sbuf_tensor("g3", [P, DT2, 16], u16))
        g4 = ctx.enter_context(nc.sbuf_tensor("g4", [P, DT2, 8], u16))
        lo_s = ctx.enter_context(nc.sbuf_tensor("lo_s", [P, DT2, 8], u16))
        hi_s = ctx.enter_context(nc.sbuf_tensor("hi_s", [P, DT2, 8], u16))
        s_s = ctx.enter_context(nc.sbuf_tensor("s_s", [P, DT2, 8], u16))
        s2 = ctx.enter_context(nc.sbuf_tensor("s2", [P, DT2, 4], u16))
        s3 = ctx.enter_context(nc.sbuf_tensor("s3", [P, DT2, 2], u16))
        actf = ctx.enter_context(nc.sbuf_tensor("actf", [P, DT2], f32))
        ident = ctx.enter_context(nc.sbuf_tensor("ident", [P, P], f32))
        bias_sb = ctx.enter_context(nc.sbuf_tensor("bias_sb", [DT2, P], f32))
        out_sb = ctx.enter_context(nc.sbuf_tensor("out_sb", [DT2, P], u8))
        wring = [
            ctx.enter_context(
                nc.sbuf_tensor(f"wring{s}", [P, PC_KT, OUT], f8)
            )
            for s in range(NSLOT)
        ]
        xpe_sb = ctx.enter_context(nc.sbuf_tensor("xpe_sb", [P, DP, KT, 16], f8))
        bpe_sb = ctx.enter_context(nc.sbuf_tensor("bpe_sb", [DP, OUT], f32))
        ope_sb = ctx.enter_context(nc.sbuf_tensor("ope_sb", [DP, OUT], u8))

        block = ctx.enter_context(nc.Block())
        wsem = [ctx.enter_context(nc.semaphore(f"wsem{d}")) for d in range(DS)]
        xsem = ctx.enter_context(nc.semaphore("xsem"))
        bias_sem = ctx.enter_context(nc.semaphore("bias_sem"))
        xpe_sem = ctx.enter_context(nc.semaphore("xpe_sem"))
        bpe_sem = ctx.enter_context(nc.semaphore("bpe_sem"))
        wpesem = [
            ctx.enter_context(nc.semaphore(f"wpesem{i}")) for i in range(NPIECE)
        ]
        pcons = ctx.enter_context(nc.semaphore("pcons"))
        ident_sem = ctx.enter_context(nc.semaphore("ident_sem"))
        dve_sem = ctx.enter_context(nc.semaphore("dve_sem"))
        pe_sem = ctx.enter_context(nc.semaphore("pe_sem"))
        cmp1 = ctx.enter_context(nc.semaphore("cmp1"))
        cmp2 = ctx.enter_context(nc.semaphore("cmp2"))
        out1 = ctx.enter_context(nc.semaphore("out1"))
        out2 = ctx.enter_context(nc.semaphore("out2"))

        def wpe_piece_src(i):
            dp, pc = divmod(i, KT // PC_KT)
            src = wpe[dp, pc * PC_KT : (pc + 1) * PC_KT, :, :]
            return src.rearrange("a p n -> p a n")

        def emit_wpe_dma(eng, i):
            if i >= NSLOT:
                eng.wait_ge(pcons, i - (NSLOT - 1))
            eng.dma_start(
                out=wring[i % NSLOT][:], in_=wpe_piece_src(i)
            ).then_inc(wpesem[i], 16)

        @block.sync
        def _(sp):
            emit_wpe_dma(sp, 0)
            for d in range(0, DS, 2):  # dirs 0, 2, 4
                sp.dma_start(out=wsb[:, d, :, :], in_=w[d, :, :, :]).then_inc(
                    wsem[d], 16
                )
            for i in range(2, NPIECE, 2):
                emit_wpe_dma(sp, i)
            sp.wait_ge(cmp1, 1)
            sp.dma_start(out=ope_r[:], in_=ope_sb[:]).then_inc(out1, 16)
            sp.wait_ge(cmp2, 1)
            sp.dma_start(out=out_r[:], in_=out_sb[:]).then_inc(out2, 16)
            sp.wait_ge(out1, 16)
            sp.wait_ge(out2, 16)

        @block.scalar
        def _(act):
            act.dma_start(out=xpe_sb[:], in_=xpe[:]).then_inc(xpe_sem, 16)
            act.dma_start(out=xb[:], in_=x[:]).then_inc(xsem, 16)
            emit_wpe_dma(act, 1)
            act.dma_start(out=bias_sb[:], in_=bias_r[:]).then_inc(bias_sem, 16)
            act.dma_start(out=bpe_sb[:], in_=bpe[:]).then_inc(bpe_sem, 16)
            for d in range(1, DS, 2):  # dirs 1, 3
                act.dma_start(out=wsb[:, d, :, :], in_=w[d, :, :, :]).then_inc(
                    wsem[d], 16
                )
            for i in range(3, NPIECE, 2):
                emit_wpe_dma(act, i)

        @block.gpsimd
        def _(gp):
            # Identity for the PE transpose.
            gp.memset(ident[:], 0.0).then_inc(ident_sem, 1)
            gp.wait_ge(ident_sem, 1)
            gp.affine_select(
                out=ident[:],
                in_=ident[:],
                compare_op=Alu.not_equal,
                fill=1.0,
                base=0,
                pattern=[[-1, P]],
                channel_multiplier=1,
            ).then_inc(ident_sem, 1)

        @block.tensor
        def _(pe):
            pe.wait_ge(xpe_sem, 16)
            for i in range(NPIECE):
                dp, pc = divmod(i, KT // PC_KT)
                pe.wait_ge(wpesem[i], 16)
                # DoubleRow: one matmul contracts both k-tiles of the piece
                # (2 fp8 weights per cell; ring slot layout [K, 2, N] is
                # exactly the interleave DR expects).
                lhsT = xpe_sb[:, dp, pc * PC_KT : (pc + 1) * PC_KT, :]
                first = i == 0
                last = i == NPIECE - 1
                for nn in range(NCH):
                    mm = pe.matmul(
                        out=psum_pe[:, nn * 512 : (nn + 1) * 512],
                        lhsT=lhsT,
                        rhs=wring[i % NSLOT][:, :, nn * 512 : (nn + 1) * 512],
                        start=first,
                        stop=last,
                        perf_mode=mybir.MatmulPerfMode.DoubleRow,
                    )
                mm.then_inc(pcons, 1)
            pe.wait_ge(ident_sem, 2)
            pe.wait_ge(dve_sem, 1)
            pe.transpose(psum_t[:], actf[:], ident[:]).then_inc(pe_sem, 1)

        @block.vector
        def _(dve):
            dve.wait_ge(xsem, 16)
            for d in range(DS):
                dve.wait_ge(wsem[d], 16)
                xa = xb[:, d, :]
                xrep = bass.AP(
                    tensor=xa.tensor,
                    offset=xa.offset,
                    ap=[list(xa.ap)[0], [0, NT], [1, K]],
                )
                wd = wsb[:, d, :, :]
                dve.tensor_tensor(
                    out=u_s[:], in0=wd, in1=xrep, op=Alu.bitwise_and
                )
                dve.tensor_scalar(
                    out=t_s[:],
                    in0=u_s[:],
                    scalar1=1,
                    scalar2=0x5555,
                    op0=Alu.logical_shift_right,
                    op1=Alu.bitwise_and,
                )
                dve.tensor_tensor(
                    out=v1_s[:], in0=u_s[:], in1=t_s[:], op=Alu.subtract
                )
                dve.tensor_scalar(
                    out=t2_s[:],
                    in0=v1_s[:],
                    scalar1=2,
                    scalar2=0x3333,
                    op0=Alu.logical_shift_right,
                    op1=Alu.bitwise_and,
                )
                dve.tensor_scalar(
                    out=m_s[:], in0=v1_s[:], scalar1=0x3333, scalar2=None,
                    op0=Alu.bitwise_and,
                )
                dve.tensor_tensor(
                    out=v2[:, d * NT : (d + 1) * NT, :],
                    in0=m_s[:],
                    in1=t2_s[:],
                    op=Alu.add,
                )
            # Fold tree across all (dir, tile) columns.
            dve.tensor_tensor(
                out=f1[:], in0=v2[:, :, 0:64], in1=v2[:, :, 64:128], op=Alu.add
            )
            dve.tensor_scalar(
                out=h_s[:],
                in0=f1[:],
                scalar1=4,
                scalar2=0x0F0F,
                op0=Alu.logical_shift_right,
                op1=Alu.bitwise_and,
            )
            dve.tensor_scalar(
                out=f1[:], in0=f1[:], scalar1=0x0F0F, scalar2=None,
                op0=Alu.bitwise_and,
            )
            dve.tensor_tensor(
                out=g_s[:], in0=f1[:], in1=h_s[:], op=Alu.add
            )
            dve.tensor_tensor(
                out=g2[:], in0=g_s[:, :, 0:32], in1=g_s[:, :, 32:64], op=Alu.add
            )
            dve.tensor_tensor(
                out=g3[:], in0=g2[:, :, 0:16], in1=g2[:, :, 16:32], op=Alu.add
            )
            dve.tensor_tensor(
                out=g4[:], in0=g3[:, :, 0:8], in1=g3[:, :, 8:16], op=Alu.add
            )
            dve.tensor_scalar(
                out=lo_s[:], in0=g4[:], scalar1=0x00FF, scalar2=None,
                op0=Alu.bitwise_and,
            )
            dve.tensor_scalar(
                out=hi_s[:], in0=g4[:], scalar1=8, scalar2=None,
                op0=Alu.logical_shift_right,
            )
            dve.tensor_tensor(
                out=s_s[:], in0=hi_s[:], in1=lo_s[:], op=Alu.add
            )
            dve.tensor_tensor(
                out=s2[:], in0=s_s[:, :, 0:4], in1=s_s[:, :, 4:8], op=Alu.add
            )
            dve.tensor_tensor(
                out=s3[:], in0=s2[:, :, 0:2], in1=s2[:, :, 2:4], op=Alu.add
            )
            # Final fold emits fp32 directly (ALU is fp32-internal).
            actv = actf[:].rearrange("p (a z) -> p a z", z=1)
            dve.tensor_tensor(
                out=actv, in0=s3[:, :, 0:1], in1=s3[:, :, 1:2], op=Alu.add
            ).then_inc(dve_sem, 1)
            # PE-direction compare while the PE transposes the SWAR act.
            dve.wait_ge(pcons, NPIECE)
            dve.wait_ge(bpe_sem, 16)
            dve.tensor_tensor(
                out=ope_sb[:], in0=psum_pe[0:DP, :], in1=bpe_sb[:], op=Alu.is_gt
            ).then_inc(cmp1, 1)
            dve.wait_ge(pe_sem, 1)
            dve.wait_ge(bias_sem, 16)
            dve.tensor_tensor(
                out=out_sb[:], in0=psum_t[:], in1=bias_sb[:], op=Alu.is_gt
            ).then_inc(cmp2, 1)

    return nc


_prog = None


def _get_prog() -> bass.Bass:
    global _prog
    if _prog is None:
        _prog = build_program()
    return _prog


def _pack_bits_u16(bits_u8: np.ndarray) -> np.ndarray:
    """[..., N] 0/1 uint8 -> [..., N//16] uint16, LSB-first."""
    b8 = np.packbits(
        bits_u8.reshape(*bits_u8.shape[:-1], -1, 8), axis=-1, bitorder="little"
    )
    return (
        np.ascontiguousarray(b8)
        .reshape(*bits_u8.shape[:-1], bits_u8.shape[-1] // 8)
        .view("<u2")
    )


def make_in_maps(weight_noise, x, bias_noise):
    np_f8 = mybir.dt.np(mybir.dt.float8e4)
    wb = np.ascontiguousarray(weight_noise).astype(np.uint8)
    xb_ = np.ascontiguousarray(x).astype(np.uint8)
    bf = np.ascontiguousarray(bias_noise, dtype=np.float32)

    in_maps = []
    for c in range(N_CORES):
        g0 = c * D
        # SWAR dirs: bit-pack + pre-transpose to [d, p, ctile, k]
        ws = _pack_bits_u16(wb[g0 : g0 + DS])  # [DS, OUT, K]
        ws = np.ascontiguousarray(
            ws.reshape(DS, NT, P, K).transpose(0, 2, 1, 3)
        )
        xs = _pack_bits_u16(xb_[g0 : g0 + DS])  # [DS, K]
        xbc = np.ascontiguousarray(
            np.broadcast_to(xs[None, :, :], (P, DS, K))
        )
        # PE dirs: fp8 W^T laid out [dp, kt, p(i), o]
        wp8 = (wb[g0 + DS : g0 + D] * np.uint8(0x38)).view(np_f8)  # [DP,OUT,IN]
        wpe = np.ascontiguousarray(
            wp8.transpose(0, 2, 1).reshape(DP, KT, P, OUT)
        )
        # stationary x tiles [p, dp, kt, m]: column m==dp holds x bits
        xpe = np.zeros((P, DP, KT, 16), dtype=np.uint8)
        for dp in range(DP):
            xpe[:, dp, :, dp] = (
                xb_[g0 + DS + dp].reshape(KT, P).T * np.uint8(0x38)
            )
        in_maps.append(
            {
                "w": ws,
                "x": xbc,
                "b": bf[g0 : g0 + DS],
                "wpe": wpe,
                "xpe": xpe.view(np_f8),
                "bpe": bf[g0 + DS : g0 + D],
            }
        )
    return in_maps


def kernel(**inputs) -> np.ndarray:
    nc = _get_prog()
    in_maps = make_in_maps(
        inputs["weight_noise"], inputs["x"], inputs["bias_noise"]
    )
    res = run_bass_kernel_spmd(nc, in_maps, list(range(N_CORES)))
    outs = [res.results[c]["o"] for c in range(N_CORES)]
    return np.concatenate(outs, axis=0).astype(bool)


# revision 42
# speedup vs baseline: 1.1899x; 1.1899x over previous
"""Trainium2 Bass kernel for nn_BinarizedLinear (ES population binary matvec).

Computes, for each direction d: out[d, o] = (sum_i W[d,o,i] * x[d,i]) > bias[d,o]
with W in {0,1} (f32), x in {0,1} (bool), bias f32.

Hybrid strategy (memory-bound problem -> shrink the stream, use every engine):
  - 8 directions per core.  Five stream as 1-bit-packed uint16 words
    (0.5 MiB/dir) and are popcounted on the DVE with the classic SWAR
    ladder (bitwise ops are raw bits; add/sub run through the fp32-internal
    ALU and stay exact below 2^24).  Three stream as fp8 (4 MiB/dir) and
    run as matvecs on the otherwise-idle PE: W^T is the moving operand,
    x sits as a [128, 3] stationary whose only nonzero column is the
    direction index, so each direction's activations accumulate into its
    own PSUM partition -> the compare + store are contiguous, no transpose.
    The 5/3 split balances DVE time against DMA bytes (PE fp8 runs at
    bf16 speed; products are 0/1 and PSUM accumulates fp32-exact).
  - SWAR ladder per direction (uint16 elements, 2048 els/partition per
    instruction -- larger flat instructions drop below the 2x/4x DVE modes):
       u  = w & x
       v1 = u - ((u >> 1) & 0x5555)              crumb pops <= 2
       v2 = (v1 & 0x3333) + ((v1>>2) & 0x3333)   nibble pops <= 4
    then a fold tree along each row's 128 words: fold64, nibble->byte
    merge via masked adds, fold to 8, byte->word merge, fold to 1 (the
    final add emits fp32).  PE-transpose, is_gt vs bias, store bools.
"""

from contextlib import ExitStack

import numpy as np

import concourse.bass as bass
from concourse import mybir
from concourse.bass_utils import run_bass_kernel_spmd

N_CORES = 8
D_TOT, OUT, IN = 64, 2048, 2048
D = D_TOT // N_CORES  # 8 directions per core
DS = 5                # SWAR (bit-packed) directions per core
DP = D - DS           # PE (fp8 matvec) directions per core
P = 128
NT = OUT // P         # 16 o-tiles per direction
K = IN // 16          # 128 packed uint16 words per row
DT2 = DS * NT         # 80 (dir, tile) columns on the SWAR path
KT = IN // P          # 16 contraction tiles per PE direction
PC_KT = 2             # K-tiles per PE weight DMA piece
NPIECE = DP * KT // PC_KT  # 24 pieces
NSLOT = 12            # fp8 weight ring slots
NCH = 4               # 512-wide PSUM chunks per row


def build_program() -> bass.Bass:
    f32 = mybir.dt.float32
    u16 = mybir.dt.uint16
    u8 = mybir.dt.uint8
    f8 = mybir.dt.float8e4
    Alu = mybir.AluOpType

    nc = bass.Bass()
    w = nc.declare_dram_parameter("w", [DS, P, NT, K], u16, isOutput=False)
    x = nc.declare_dram_parameter("x", [P, DS, K], u16, isOutput=False)
    b = nc.declare_dram_parameter("b", [DS, OUT], f32, isOutput=False)
    wpe = nc.declare_dram_parameter(
        "wpe", [DP, KT, P, OUT], f8, isOutput=False
    )
    xpe = nc.declare_dram_parameter("xpe", [P, DP, KT, 16], f8, isOutput=False)
    bpe = nc.declare_dram_parameter("bpe", [DP, OUT], f32, isOutput=False)
    o = nc.declare_dram_parameter("o", [D, OUT], u8, isOutput=True)

    # [80, 128] views of SWAR bias/out matching the post-transpose layout:
    # partition j = d*16 + c, free f = p  ->  flat offset j*128 + f.
    bias_r = b[:].rearrange("d (h f) -> (d h) f", f=P)
    out_r = o[0:DS, :].rearrange("d (h f) -> (d h) f", f=P)
    ope_r = o[DS:D, :]

    psum_t = nc.alloc_psum_tensor("psum_t", [DT2, P], f32)
    psum_pe = nc.alloc_psum_tensor("psum_pe", [16, OUT], f32)

    with ExitStack() as ctx:
        wsb = ctx.enter_context(nc.sbuf_tensor("wsb", [P, DS, NT, K], u16))
        xb = ctx.enter_context(nc.sbuf_tensor("xb", [P, DS, K], u16))
        u_s = ctx.enter_context(nc.sbuf_tensor("u_s", [P, NT, K], u16))
        t_s = ctx.enter_context(nc.sbuf_tensor("t_s", [P, NT, K], u16))
        v1_s = ctx.enter_context(nc.sbuf_tensor("v1_s", [P, NT, K], u16))
        t2_s = ctx.enter_context(nc.sbuf_tensor("t2_s", [P, NT, K], u16))
        m_s = ctx.enter_context(nc.sbuf_tensor("m_s", [P, NT, K], u16))
        v2 = ctx.enter_context(nc.sbuf_tensor("v2", [P, DT2, K], u16))
        f1 = ctx.enter_context(nc.sbuf_tensor("f1", [P, DT2, 64], u16))
        h_s = ctx.enter_context(nc.sbuf_tensor("h_s", [P, DT2, 64], u16))
        g_s = ctx.enter_context(nc.sbuf_tensor("g_s", [P, DT2, 64], u16))
        g2 = ctx.enter_context(nc.sbuf_tensor("g2", [P, DT2, 32], u16))
        g3 = ctx.enter_context(nc.sbuf_tensor("g3", [P, DT2, 16], u16))
        g4 = ctx.enter_context(nc.sbuf_tensor("g4", [P, DT2, 8], u16))
        lo_s = ctx.enter_context(nc.sbuf_tensor("lo_s", [P, DT2, 8], u16))
        hi_s = ctx.enter_context(nc.sbuf_tensor("hi_s", [P, DT2, 8], u16))
        s_s = ctx.enter_context(nc.sbuf_tensor("s_s", [P, DT2, 8], u16))
        s2 = ctx.enter_context(nc.sbuf_tensor("s2", [P, DT2, 4], u16))
        s3 = ctx.enter_context(nc.sbuf_tensor("s3", [P, DT2, 2], u16))
        actf = ctx.enter_context(nc.sbuf_tensor("actf", [P, DT2], f32))
        ident = ctx.enter_context(nc.sbuf_tensor("ident", [P, P], f32))
        bias_sb = ctx.enter_context(nc.sbuf_tensor("bias_sb", [DT2, P], f32))
        out_sb = ctx.enter_context(nc.sbuf_tensor("out_sb", [DT2, P], u8))
        wring = [
            ctx.enter_context(
                nc.sbuf_tensor(f"wring{s}", [P, PC_KT, OUT], f8)
            )
            for s in range(NSLOT)
        ]
        xpe_sb = ctx.enter_context(nc.sbuf_tensor("xpe_sb", [P, DP, KT, 16], f8))
        bpe_sb = ctx.enter_context(nc.sbuf_tensor("bpe_sb", [DP, OUT], f32))
        ope_sb = ctx.enter_context(nc.sbuf_tensor("ope_sb", [DP, OUT], u8))

        block = ctx.enter_context(nc.Block())
        wsem = [ctx.enter_context(nc.semaphore(f"wsem{d}")) for d in range(DS)]
        xsem = ctx.enter_context(nc.semaphore("xsem"))
        bias_sem = ctx.enter_context(nc.semaphore("bias_sem"))
        xpe_sem = ctx.enter_context(nc.semaphore("xpe_sem"))
        bpe_sem = ctx.enter_context(nc.semaphore("bpe_sem"))
        wpesem = [
            ctx.enter_context(nc.semaphore(f"wpesem{i}")) for i in range(NPIECE)
        ]
        pcons = ctx.enter_context(nc.semaphore("pcons"))
        ident_sem = ctx.enter_context(nc.semaphore("ident_sem"))
        dve_sem = ctx.enter_context(nc.semaphore("dve_sem"))
        pe_sem = ctx.enter_context(nc.semaphore("pe_sem"))
        cmp1 = ctx.enter_context(nc.semaphore("cmp1"))
        cmp2 = ctx.enter_context(nc.semaphore("cmp2"))
        out1 = ctx.enter_context(nc.semaphore("out1"))
        out2 = ctx.enter_context(nc.semaphore("out2"))

        def wpe_piece_src(i):
            dp, pc = divmod(i, KT // PC_KT)
            src = wpe[dp, pc * PC_KT : (pc + 1) * PC_KT, :, :]
            return src.rearrange("a p n -> p a n")

        def emit_wpe_dma(eng, i):
            if i >= NSLOT:
                eng.wait_ge(pcons, i - (NSLOT - 1))
            eng.dma_start(
                out=wring[i % NSLOT][:], in_=wpe_piece_src(i)
            ).then_inc(wpesem[i], 16)

        @block.sync
        def _(sp):
            emit_wpe_dma(sp, 0)
            for d in range(0, DS, 2):  # dirs 0, 2, 4
                sp.dma_start(out=wsb[:, d, :, :], in_=w[d, :, :, :]).then_inc(
                    wsem[d], 16
                )
            for i in range(2, NPIECE, 2):
                emit_wpe_dma(sp, i)
            sp.wait_ge(cmp1, 1)
            sp.dma_start(out=ope_r[:], in_=ope_sb[:]).then_inc(out1, 16)
            sp.wait_ge(cmp2, 1)
            sp.dma_start(out=out_r[:], in_=out_sb[:]).then_inc(out2, 16)
            sp.wait_ge(out1, 16)
            sp.wait_ge(out2, 16)

        @block.scalar
        def _(act):
            act.dma_start(out=xpe_sb[:], in_=xpe[:]).then_inc(xpe_sem, 16)
            act.dma_start(out=xb[:], in_=x[:]).then_inc(xsem, 16)
            emit_wpe_dma(act, 1)
            act.dma_start(out=bias_sb[:], in_=bias_r[:]).then_inc(bias_sem, 16)
            act.dma_start(out=bpe_sb[:], in_=bpe[:]).then_inc(bpe_sem, 16)
            for d in range(1, DS, 2):  # dirs 1, 3
                act.dma_start(out=wsb[:, d, :, :], in_=w[d, :, :, :]).then_inc(
                    wsem[d], 16
                )
            for i in range(3, NPIECE, 2):
                emit_wpe_dma(act, i)

        @block.gpsimd
        def _(gp):
            # Identity for the PE transpose.
            gp.memset(ident[:], 0.0).then_inc(ident_sem, 1)
            gp.wait_ge(ident_sem, 1)
            gp.affine_select(
                out=ident[:],
                in_=ident[:],
                compare_op=Alu.not_equal,
                fill=1.0,
                base=0,
                pattern=[[-1, P]],
                channel_multiplier=1,
            ).then_inc(ident_sem, 1)

        @block.tensor
        def _(pe):
            pe.wait_ge(xpe_sem, 16)
            for i in range(NPIECE):
                dp, pc = divmod(i, KT // PC_KT)
                pe.wait_ge(wpesem[i], 16)
                # DoubleRow: one matmul contracts both k-tiles of the piece
                # (2 fp8 weights per cell; ring slot layout [K, 2, N] is
                # exactly the interleave DR expects).
                lhsT = xpe_sb[:, dp, pc * PC_KT : (pc + 1) * PC_KT, :]
                first = i == 0
                last = i == NPIECE - 1
                for nn in range(NCH):
                    mm = pe.matmul(
                        out=psum_pe[:, nn * 512 : (nn + 1) * 512],
                        lhsT=lhsT,
                        rhs=wring[i % NSLOT][:, :, nn * 512 : (nn + 1) * 512],
                        start=first,
                        stop=last,
                        perf_mode=mybir.MatmulPerfMode.DoubleRow,
                    )
                mm.then_inc(pcons, 1)
            pe.wait_ge(ident_sem, 2)
            pe.wait_ge(dve_sem, 1)
            pe.transpose(psum_t[:], actf[:], ident[:]).then_inc(pe_sem, 1)

        @block.vector
        def _(dve):
            dve.wait_ge(xsem, 16)
            for d in range(DS):
                dve.wait_ge(wsem[d], 16)
                xa = xb[:, d, :]
                xrep = bass.AP(
                    tensor=xa.tensor,
                    offset=xa.offset,
                    ap=[list(xa.ap)[0], [0, NT], [1, K]],
                )
                wd = wsb[:, d, :, :]
                dve.tensor_tensor(
                    out=u_s[:], in0=wd, in1=xrep, op=Alu.bitwise_and
                )
                dve.tensor_scalar(
                    out=t_s[:],
                    in0=u_s[:],
                    scalar1=1,
                    scalar2=0x5555,
                    op0=Alu.logical_shift_right,
                    op1=Alu.bitwise_and,
                )
                dve.tensor_tensor(
                    out=v1_s[:], in0=u_s[:], in1=t_s[:], op=Alu.subtract
                )
                dve.tensor_scalar(
                    out=t2_s[:],
                    in0=v1_s[:],
                    scalar1=2,
                    scalar2=0x3333,
                    op0=Alu.logical_shift_right,
                    op1=Alu.bitwise_and,
                )
                dve.tensor_scalar(
                    out=m_s[:], in0=v1_s[:], scalar1=0x3333, scalar2=None,
                    op0=Alu.bitwise_and,
                )
                dve.tensor_tensor(
                    out=v2[:, d * NT : (d + 1) * NT, :],
                    in0=m_s[:],
                    in1=t2_s[:],
                    op=Alu.add,
                )
            # Fold tree across all (dir, tile) columns.
            dve.tensor_tensor(
                out=f1[:], in0=v2[:, :, 0:64], in1=v2[:, :, 64:128], op=Alu.add
            )
            dve.tensor_scalar(
                out=h_s[:],
                in0=f1[:],
                scalar1=4,
                scalar2=0x0F0F,
                op0=Alu.logical_shift_right,
                op1=Alu.bitwise_and,
            )
            dve.tensor_scalar(
                out=f1[:], in0=f1[:], scalar1=0x0F0F, scalar2=None,
                op0=Alu.bitwise_and,
            )
            dve.tensor_tensor(
                out=g_s[:], in0=f1[:], in1=h_s[:], op=Alu.add
            )
            dve.tensor_tensor(
                out=g2[:], in0=g_s[:, :, 0:32], in1=g_s[:, :, 32:64], op=Alu.add
            )
            dve.tensor_tensor(
                out=g3[:], in0=g2[:, :, 0:16], in1=g2[:, :, 16:32], op=Alu.add
            )
            dve.tensor_tensor(
                out=g4[:], in0=g3[:, :, 0:8], in1=g3[:, :, 8:16], op=Alu.add
            )
            dve.tensor_scalar(
                out=lo_s[:], in0=g4[:], scalar1=0x00FF, scalar2=None,
                op0=Alu.bitwise_and,
            )
            dve.tensor_scalar(
                out=hi_s[:], in0=g4[:], scalar1=8, scalar2=None,
                op0=Alu.logical_shift_right,
            )
            dve.tensor_tensor(
                out=s_s[:], in0=hi_s[:], in1=lo_s[:], op=Alu.add
            )
            dve.tensor_tensor(
                out=s2[:], in0=s_s[:, :, 0:4], in1=s_s[:, :, 4:8], op=Alu.add
            )
            dve.tensor_tensor(
                out=s3[:], in0=s2[:, :, 0:2], in1=s2[:, :, 2:4], op=Alu.add
            )
            # Final fold emits fp32 directly (ALU is fp32-internal).
            actv = actf[:].rearrange("p (a z) -> p a z", z=1)
            dve.tensor_tensor(
                out=actv, in0=s3[:, :, 0:1], in1=s3[:, :, 1:2], op=Alu.add
            ).then_inc(dve_sem, 1)
            # PE-direction compare while the PE transposes the SWAR act.
            dve.wait_ge(pcons, NPIECE)
            dve.wait_ge(bpe_sem, 16)
            dve.tensor_tensor(
                out=ope_sb[:], in0=psum_pe[0:DP, :], in1=bpe_sb[:],
                op=Alu.is_gt,
            ).then_inc(cmp1, 1)
            dve.wait_ge(pe_sem, 1)
            dve.wait_ge(bias_sem, 16)
            dve.tensor_tensor(
                out=out_sb[:], in0=psum_t[:], in1=bias_sb[:], op=Alu.is_gt
            ).then_inc(cmp2, 1)

    return nc


_prog = None


def _get_prog() -> bass.Bass:
    global _prog
    if _prog is None:
        _prog = build_program()
    return _prog


def _pack_bits_u16(bits_u8: np.ndarray) -> np.ndarray:
    """[..., N] 0/1 uint8 -> [..., N//16] uint16, LSB-first."""
    b8 = np.packbits(
        bits_u8.reshape(*bits_u8.shape[:-1], -1, 8), axis=-1, bitorder="little"
    )
    return (
        np.ascontiguousarray(b8)
        .reshape(*bits_u8.shape[:-1], bits_u8.shape[-1] // 8)
        .view("<u2")
    )


def make_in_maps(weight_noise, x, bias_noise):
    np_f8 = mybir.dt.np(mybir.dt.float8e4)
    wb = np.ascontiguousarray(weight_noise).astype(np.uint8)
    xb_ = np.ascontiguousarray(x).astype(np.uint8)
    bf = np.ascontiguousarray(bias_noise, dtype=np.float32)

    in_maps = []
    for c in range(N_CORES):
        g0 = c * D
        # SWAR dirs: bit-pack + pre-transpose to [d, p, ctile, k]
        ws = _pack_bits_u16(wb[g0 : g0 + DS])  # [DS, OUT, K]
        ws = np.ascontiguousarray(
            ws.reshape(DS, NT, P, K).transpose(0, 2, 1, 3)
        )
        xs = _pack_bits_u16(xb_[g0 : g0 + DS])  # [DS, K]
        xbc = np.ascontiguousarray(
            np.broadcast_to(xs[None, :, :], (P, DS, K))
        )
        # PE dirs: fp8 W^T laid out [dp, kt, p(i), o]
        wp8 = (wb[g0 + DS : g0 + D] * np.uint8(0x38)).view(np_f8)  # [DP,OUT,IN]
        wpe = np.ascontiguousarray(
            wp8.transpose(0, 2, 1).reshape(DP, KT, P, OUT)
        )
        # stationary x tiles [p, dp, kt, m]: column m==dp holds x bits
        xpe = np.zeros((P, DP, KT, 16), dtype=np.uint8)
        for dp in range(DP):
            xpe[:, dp, :, dp] = (
                xb_[g0 + DS + dp].reshape(KT, P).T * np.uint8(0x38)
            )
        in_maps.append(
            {
                "w": ws,
                "x": xbc,
                "b": bf[g0 : g0 + DS],
                "wpe": wpe,
                "xpe": xpe.view(np_f8),
                "bpe": bf[g0 + DS : g0 + D],
            }
        )
    return in_maps


def kernel(**inputs) -> np.ndarray:
    nc = _get_prog()
    in_maps = make_in_maps(
        inputs["weight_noise"], inputs["x"], inputs["bias_noise"]
    )
    res = run_bass_kernel_spmd(nc, in_maps, list(range(N_CORES)))
    outs = [res.results[c]["o"] for c in range(N_CORES)]
    return np.concatenate(outs, axis=0).astype(bool)
